# revision 49
# baseline (speedup 1.0000x reference)
"""Trainium2 Bass kernel for nn_AnalogLayer.

Math (see reference):
    A[p, m] built from cos/sin of (-2*pi/256 * p_values[p%64] * (m%256)),
    y[bn, :] = A @ x[bn, :]  for each of the batch*32 rows of length 512.

Strategy: pure data-parallel over batch across 8 NeuronCores. Per core the
shard is viewed as [16384, 512] rows; output is produced transposed
([128, 16384] bf16) and fixed up on the host during the gather/unshard step.
The kernel is HBM-bound (33.5MB f32 in + 4.2MB bf16 out per core at ~358
GB/s/core ~= 105us); the design keeps the DMA stream saturated end-to-end.

Host side: A^T is precomputed from the tiny p_values and passed in
replicated per 32-row strip and even/odd split
(atrep[32a+r, j2, h, p] = A[p, 64*j2 + 2r + h]).

Per-core pipeline (per super-group of 2048 rows, in ~1MB chunks):
  - SWDGE cast-DMA x f32(HBM) -> bf16(SBUF) [128, 4, 512] chunks, with
    partition = row-block ("(p t)"): one contiguous 8KB read run per
    partition per chunk (4x fewer descriptors than the strided layout)
  - VectorE StreamTranspose of bf16 PAIRS viewed as f32 via bitcast
    (32x32 blocks of pairs): stream transpose moves 1 elem/lane/cycle
    regardless of dtype, so pairing halves DVE time (75us -> 41us).
    Partition 32a+r then holds m in {64*j2+2r, 64*j2+2r+1}.
  - 8 j2-blocks x 2 parities x 4 row-strips of K=32 matmuls
    (tile_position=(32a,0), quad-concurrent), lhsT = the matching
    even/odd A^T slice, rhs = stride-2 columns; same instruction count,
    N = nt*32 <= 512 per PSUM bank, one bank per strip
  - ScalarE drains PSUM -> SBUF bf16 with permuted APs (undoing the
    "(p t)" row permutation); out-DMA per super on the sync HWDGE ring
  - taper [2048x7, 1024, 512, 512] with final chunks [2,1,1] keeps the
    post-last-input serial chain (transpose+matmul+drain+out) short

Lessons encoded here (measured on HW): instruction count costs ~6-7ns
each in init TENSOR_LOAD + barrier + refill, so big-N matmuls matter;
fine-grained out-DMAs degrade the input stream's HBM efficiency; strided
matmul rhs APs collapse PE streaming; HWDGE-f32 + engine casts lose to
SWDGE cast-DMA; device timing has ~15% contention episodes (measure
min-of-5).
"""

import math

import numpy as np

import concourse.bacc as bacc
import concourse.bass as bass
import concourse.masks as masks
import concourse.mybir as mybir
import concourse.tile as tile
from concourse.bass_utils import run_bass_kernel_spmd

N_CORES = 8
BATCH = 4096
SHARD = BATCH // N_CORES  # 512 batch rows per core
NBLK = 32                 # blocks per batch row
M2 = 512                  # contraction dim (2*M)
P2 = 128                  # output dim (2*P)
BN = SHARD * NBLK         # 16384 logical rows per core
import os

SUPER = int(os.environ.get("K_SUPER", "2048"))  # rows per super-group
QN = int(os.environ.get("K_QN", "4"))           # t-tiles per load chunk
XFBUFS = int(os.environ.get("K_XFBUFS", "12"))
TAPER = os.environ.get("K_TAPER", "1") == "1"
NT = SUPER // 128
NSG = BN // SUPER
NJQ = 16                  # j-chunks (contraction 512 / 32)

F32 = mybir.dt.float32
BF16 = mybir.dt.bfloat16
PI = math.pi


def build_kernel():
    nc = bacc.Bacc("TRN2", target_bir_lowering=False)
    x_d = nc.declare_dram_parameter("x", [BN, M2], F32, isOutput=False)
    at_d = nc.declare_dram_parameter("at", [128, 8, 2, 128], BF16, isOutput=False)
    out_d = nc.declare_dram_parameter("out", [P2, BN], BF16, isOutput=True)

    with tile.TileContext(nc) as tc:
        with (
            tc.tile_pool(name="const", bufs=1) as cpool,
            tc.tile_pool(name="xf", bufs=2) as xf_pool,
            tc.tile_pool(name="vt", bufs=int(os.environ.get("K_VBUFS", "3"))) as vt_pool,
            tc.tile_pool(name="ysb", bufs=3) as y_pool,
        ):
            # A^T is host-precomputed (tiny: derived from the 64-entry
            # p_values) and passed in replicated and even/odd-split:
            # atrep[32a+r, j2, h, p] = A[p, 64*j2 + 2r + h]. The split
            # matches the f32-pair transpose trick (see main loop).
            ATrep = cpool.tile([128, 8, 2, 128], BF16)
            nc.scalar.dma_start(ATrep[:], at_d[:])

            # ---------------- main loop ----------------
            # Per super-group: SWDGE cast-DMA x f32->bf16 in ~1MB chunks ->
            # DVE 32x32 transpose per chunk -> 16j x 4a quad-32 matmuls with
            # N = nt*32 (large N amortizes LDWEIGHTS + keeps the instruction
            # count low; instruction refill rides DMA row 14) -> ScalarE
            # drains -> one coarse out-DMA per super on the sync ring
            # (fine-grained out writes interleave badly with the input read
            # stream at HBM). Single small final super keeps the tail short.
            ps_cm = tc.tile_pool(
                name="ps", bufs=(4 if SUPER == 4096 else 8), space="PSUM"
            )
            ps = ps_cm.__enter__()

            if SUPER == 4096 and TAPER:
                sizes = [4096] * 3 + [2048, 1024, 512, 512]
            elif SUPER == 2048 and TAPER:
                sizes = [2048] * 7 + [1024, 512, 512]
            else:
                sizes = [SUPER] * NSG
            assert sum(sizes) == BN
            # Full-size chunks in the body; the final super uses small
            # chunks so the DVE transpose backlog at input-end is at most
            # one ~0.6us instruction. (Instruction count is kept low: init
            # TENSOR_LOAD + barrier + refill cost ~6-7ns per instruction.)
            chunk_splits = {sg: [4] * (r // 512) for sg, r in enumerate(sizes)}
            # Small chunks at the tail: the post-last-input chain is then
            # one small transpose + a short matmul phase + drain + out.
            chunk_splits[len(sizes) - 1] = [2, 1, 1]
            row0 = 0
            nchunk = 0
            for sg, rows_n in enumerate(sizes):
                nt = rows_n // 128
                rows = slice(row0, row0 + rows_n)
                # Body supers (uniform qn=4 chunks) load with partition =
                # row-block ("(p t)"): one contiguous 8KB run per partition
                # per chunk (4x fewer DMA descriptors, faster SWDGE
                # emission). This permutes rows within the chunk; the drain
                # APs undo it. Taper supers use the plain layout.
                contig = chunk_splits[sg] == [4] * (rows_n // 512)
                v = vt_pool.tile([128, nt, 512], BF16, tag="v", name=f"v{sg}")
                t0 = 0
                for qn in chunk_splits[sg]:
                    crows = slice(row0 + t0 * 128, row0 + (t0 + qn) * 128)
                    xf = xf_pool.tile(
                        [128, QN, 512], BF16, tag="xf", name=f"xf{sg}_{t0}",
                        bufs=XFBUFS,
                    )
                    if contig:
                        x_src = x_d[crows, :].rearrange("(p t) m -> p t m", t=qn)
                    else:
                        x_src = x_d[crows, :].rearrange("(t p) m -> p t m", p=128)
                    # SWDGE cast-DMA: f32 HBM read, bf16 SBUF write
                    nc.gpsimd.dma_start(xf[:, :qn, :], x_src)
                    # Transpose bf16 PAIRS as f32 elements: halves the DVE
                    # element count (stream transpose moves 1 elem/lane/cyc
                    # regardless of dtype). The pair interleave is undone
                    # by the even/odd split of A^T in the matmul.
                    nc.vector.transpose(
                        v[:, t0 : t0 + qn, :].bitcast(F32),
                        xf[:, :qn, :].bitcast(F32),
                    )
                    t0 += qn
                    nchunk += 1
                assert t0 == nt

                yts = [
                    ps.tile([128, nt, 32], F32, tag="yt", name=f"yt{sg}_{a}")
                    for a in range(4)
                ]
                # v[32a+r, t, 64*j2 + 2c + h] = x[row(a,c,t), 64*j2+2r+h]:
                # contract per (j2, h) with the matching A^T slice; columns
                # (t, c) land in the same output-row order as before.
                vv = v[:].rearrange("p t (j2 c h) -> p t j2 c h", c=32, h=2)
                for j2 in range(8):
                    for h in range(2):
                        for a in range(4):
                            nc.tensor.matmul(
                                yts[a][:],
                                ATrep[32 * a : 32 * (a + 1), j2, h, :],
                                vv[32 * a : 32 * (a + 1), :, j2, :, h],
                                start=(j2 == 0 and h == 0),
                                stop=(j2 == 7 and h == 1),
                                tile_position=(32 * a, 0),
                            )

                # Taper supers: alternate drains onto the (by then idle)
                # Vector engine so the final drains run in parallel.
                use_vec = sg >= len(sizes) - 2 and sg % 2
                if contig:
                    # Undo the "(p t)" row permutation in the drain APs:
                    # row_local = 512*Tq + 128*a + 4*c + tr (Tq=T//4, tr=T%4)
                    ysb = y_pool.tile(
                        [128, nt // 4, 4, 32, 4], BF16, tag="ysb", name=f"ysb{sg}"
                    )
                    for a in range(4):
                        dst = ysb[:, :, a, :, :].rearrange("p q c t -> p q t c")
                        src = yts[a][:].rearrange("p (q t) c -> p q t c", t=4)
                        if use_vec:
                            nc.vector.tensor_copy(dst, src)
                        else:
                            nc.scalar.copy(dst, src)
                    out_src = ysb[:].rearrange("p q a c t -> p (q a c t)")
                else:
                    ysb = y_pool.tile(
                        [128, nt, 4, 32], BF16, tag="ysb", name=f"ysb{sg}"
                    )
                    for a in range(4):
                        if use_vec:
                            nc.vector.tensor_copy(ysb[:, :, a, :], yts[a][:])
                        else:
                            nc.scalar.copy(ysb[:, :, a, :], yts[a][:])
                    out_src = ysb[:].rearrange("p t a c -> p (t a c)")
                nc.sync.dma_start(out_d[:, rows], out_src)
                row0 += rows_n
            ps_cm.__exit__(None, None, None)

    nc.compile()
    return nc


_CACHE: dict = {}


def _get_nc():
    if "nc" not in _CACHE:
        _CACHE["nc"] = build_kernel()
    return _CACHE["nc"]


def _make_atrep(p_values):
    """atrep[32a + r, j2, h, p] = A[p, 64*j2 + 2r + h], A per the reference."""
    import ml_dtypes

    pv = np.asarray(p_values, dtype=np.float64).reshape(64, 1)
    m = np.arange(256, dtype=np.float64)
    ang = -pv * m * (2.0 * math.pi / 256.0)
    A_real, A_imag = np.cos(ang), np.sin(ang)       # [64, 256]
    A1 = np.concatenate((A_real, A_imag), axis=0)   # [128, 256]
    A2 = np.concatenate((-A_imag, A_real), axis=0)
    A = np.concatenate((A1, A2), axis=1)            # [128, 512]
    AT = A.T                                        # [512, 128] = AT[m, p]
    base = AT.reshape(8, 32, 2, 128).transpose(1, 0, 2, 3)  # [r, j2, h, p]
    rep = np.tile(base, (4, 1, 1, 1))               # [128, 8, 2, 128]
    return np.ascontiguousarray(rep.astype(ml_dtypes.bfloat16))


def _run(x, p_values, trace=False, **kw):
    nc = _get_nc()
    x = np.ascontiguousarray(x, dtype=np.float32)
    at = _make_atrep(p_values)
    in_maps = [
        {"x": x[c * SHARD : (c + 1) * SHARD].reshape(BN, M2), "at": at}
        for c in range(N_CORES)
    ]
    res = run_bass_kernel_spmd(
        nc, in_maps, core_ids=list(range(N_CORES)), trace=trace, **kw
    )
    out = np.empty((BATCH, NBLK * P2), dtype=np.float32)
    for c in range(N_CORES):
        # res is [128, 16384] bf16 = y_core^T; un-transpose during the gather
        out[c * SHARD : (c + 1) * SHARD] = (
            res.results[c]["out"].astype(np.float32).T.reshape(SHARD, NBLK * P2)
        )
    return out, res


def kernel(x, p_values):
    out, _ = _run(x, p_values)
    return out



# revision 50
# speedup vs baseline: 1.0141x; 1.0141x over previous
"""Trainium2 Bass kernel for nn_AnalogLayer.

Math (see reference):
    A[p, m] built from cos/sin of (-2*pi/256 * p_values[p%64] * (m%256)),
    y[bn, :] = A @ x[bn, :]  for each of the batch*32 rows of length 512.

Strategy: pure data-parallel over batch across 8 NeuronCores. Per core the
shard is viewed as [16384, 512] rows; output is produced transposed
([128, 16384] bf16) and fixed up on the host during the gather/unshard step.
The kernel is HBM-bound (33.5MB f32 in + 4.2MB bf16 out per core at ~358
GB/s/core ~= 105us); the design keeps the DMA stream saturated end-to-end.

Host side: A^T is precomputed from the tiny p_values and passed in
replicated per 32-row strip and even/odd split
(atrep[32a+r, j2, h, p] = A[p, 64*j2 + 2r + h]).

Per-core pipeline (per super-group of 2048 rows, in ~1MB chunks):
  - SWDGE cast-DMA x f32(HBM) -> bf16(SBUF) [128, 4, 512] chunks, with
    partition = row-block ("(p t)"): one contiguous 8KB read run per
    partition per chunk (4x fewer descriptors than the strided layout)
  - VectorE StreamTranspose of bf16 PAIRS viewed as f32 via bitcast
    (32x32 blocks of pairs): stream transpose moves 1 elem/lane/cycle
    regardless of dtype, so pairing halves DVE time (75us -> 41us).
    Partition 32a+r then holds m in {64*j2+2r, 64*j2+2r+1}.
  - 8 j2-blocks x 2 parities x 4 row-strips of K=32 matmuls
    (tile_position=(32a,0), quad-concurrent), lhsT = the matching
    even/odd A^T slice, rhs = stride-2 columns; same instruction count,
    N = nt*32 <= 512 per PSUM bank, one bank per strip
  - ScalarE drains PSUM -> SBUF bf16 with permuted APs (undoing the
    "(p t)" row permutation); out-DMA per super on the sync HWDGE ring
  - taper [2048x7, 1024, 512, 512] with final chunks [2,1,1] keeps the
    post-last-input serial chain (transpose+matmul+drain+out) short

Lessons encoded here (measured on HW): instruction count costs ~6-7ns
each in init TENSOR_LOAD + barrier + refill, so big-N matmuls matter;
fine-grained out-DMAs degrade the input stream's HBM efficiency; strided
matmul rhs APs collapse PE streaming; HWDGE-f32 + engine casts lose to
SWDGE cast-DMA; device timing has ~15% contention episodes (measure
min-of-5).
"""

import math

import numpy as np

import concourse.bacc as bacc
import concourse.bass as bass
import concourse.masks as masks
import concourse.mybir as mybir
import concourse.tile as tile
from concourse.bass_utils import run_bass_kernel_spmd

N_CORES = 8
BATCH = 4096
SHARD = BATCH // N_CORES  # 512 batch rows per core
NBLK = 32                 # blocks per batch row
M2 = 512                  # contraction dim (2*M)
P2 = 128                  # output dim (2*P)
BN = SHARD * NBLK         # 16384 logical rows per core
import os

SUPER = int(os.environ.get("K_SUPER", "2048"))  # rows per super-group
QN = int(os.environ.get("K_QN", "4"))           # t-tiles per load chunk
XFBUFS = int(os.environ.get("K_XFBUFS", "12"))
TAPER = os.environ.get("K_TAPER", "1") == "1"
NT = SUPER // 128
NSG = BN // SUPER
NJQ = 16                  # j-chunks (contraction 512 / 32)

F32 = mybir.dt.float32
BF16 = mybir.dt.bfloat16
PI = math.pi


def build_kernel():
    nc = bacc.Bacc("TRN2", target_bir_lowering=False)
    x_d = nc.declare_dram_parameter("x", [BN, M2], F32, isOutput=False)
    at_d = nc.declare_dram_parameter("at", [128, 8, 2, 128], BF16, isOutput=False)
    out_d = nc.declare_dram_parameter("out", [P2, BN], BF16, isOutput=True)

    with tile.TileContext(nc) as tc:
        with (
            tc.tile_pool(name="const", bufs=1) as cpool,
            tc.tile_pool(name="xf", bufs=2) as xf_pool,
            tc.tile_pool(name="vt", bufs=int(os.environ.get("K_VBUFS", "3"))) as vt_pool,
            tc.tile_pool(name="ysb", bufs=3) as y_pool,
        ):
            # A^T is host-precomputed (tiny: derived from the 64-entry
            # p_values) and passed in replicated and even/odd-split:
            # atrep[32a+r, j2, h, p] = A[p, 64*j2 + 2r + h]. The split
            # matches the f32-pair transpose trick (see main loop).
            ATrep = cpool.tile([128, 8, 2, 128], BF16)
            nc.scalar.dma_start(ATrep[:], at_d[:])

            # ---------------- main loop ----------------
            # Per super-group: SWDGE cast-DMA x f32->bf16 in ~1MB chunks ->
            # DVE 32x32 transpose per chunk -> 16j x 4a quad-32 matmuls with
            # N = nt*32 (large N amortizes LDWEIGHTS + keeps the instruction
            # count low; instruction refill rides DMA row 14) -> ScalarE
            # drains -> one coarse out-DMA per super on the sync ring
            # (fine-grained out writes interleave badly with the input read
            # stream at HBM). Single small final super keeps the tail short.
            ps_cm = tc.tile_pool(
                name="ps", bufs=(4 if SUPER == 4096 else 8), space="PSUM"
            )
            ps = ps_cm.__enter__()

            if SUPER == 4096 and TAPER:
                sizes = [4096] * 3 + [2048, 1024, 512, 512]
            elif SUPER == 2048 and TAPER:
                sizes = [2048] * 7 + [1024, 512, 256, 128, 128]
            else:
                sizes = [SUPER] * NSG
            assert sum(sizes) == BN
            # Full-size chunks in the body; the final supers use small
            # chunks so the DVE transpose backlog at input-end is at most
            # one ~0.3us instruction. (Instruction count is kept low: init
            # TENSOR_LOAD + barrier + refill cost ~6-7ns per instruction.)
            chunk_splits = {
                sg: ([4] * (r // 512) if r >= 512 else [r // 128])
                for sg, r in enumerate(sizes)
            }
            # Small chunks at the tail: the post-last-input chain is then
            # one small transpose + a short matmul phase + drain + out.
            if sizes[-1] == 512:
                chunk_splits[len(sizes) - 1] = [2, 1, 1]
            row0 = 0
            nchunk = 0
            for sg, rows_n in enumerate(sizes):
                nt = rows_n // 128
                rows = slice(row0, row0 + rows_n)
                # Body supers (uniform qn=4 chunks) load with partition =
                # row-block ("(p t)"): one contiguous 8KB run per partition
                # per chunk (4x fewer DMA descriptors, faster SWDGE
                # emission). This permutes rows within the chunk; the drain
                # APs undo it. Taper supers use the plain layout.
                contig = chunk_splits[sg] == [4] * (rows_n // 512)
                v = vt_pool.tile([128, nt, 512], BF16, tag="v", name=f"v{sg}")
                t0 = 0
                for qn in chunk_splits[sg]:
                    crows = slice(row0 + t0 * 128, row0 + (t0 + qn) * 128)
                    xf = xf_pool.tile(
                        [128, QN, 512], BF16, tag="xf", name=f"xf{sg}_{t0}",
                        bufs=XFBUFS,
                    )
                    if contig:
                        x_src = x_d[crows, :].rearrange("(p t) m -> p t m", t=qn)
                    else:
                        x_src = x_d[crows, :].rearrange("(t p) m -> p t m", p=128)
                    # SWDGE cast-DMA: f32 HBM read, bf16 SBUF write
                    nc.gpsimd.dma_start(xf[:, :qn, :], x_src)
                    # Transpose bf16 PAIRS as f32 elements: halves the DVE
                    # element count (stream transpose moves 1 elem/lane/cyc
                    # regardless of dtype). The pair interleave is undone
                    # by the even/odd split of A^T in the matmul.
                    nc.vector.transpose(
                        v[:, t0 : t0 + qn, :].bitcast(F32),
                        xf[:, :qn, :].bitcast(F32),
                    )
                    t0 += qn
                    nchunk += 1
                assert t0 == nt

                yts = [
                    ps.tile([128, nt, 32], F32, tag="yt", name=f"yt{sg}_{a}")
                    for a in range(4)
                ]
                # v[32a+r, t, 64*j2 + 2c + h] = x[row(a,c,t), 64*j2+2r+h]:
                # contract per (j2, h) with the matching A^T slice; columns
                # (t, c) land in the same output-row order as before.
                vv = v[:].rearrange("p t (j2 c h) -> p t j2 c h", c=32, h=2)
                for j2 in range(8):
                    for h in range(2):
                        for a in range(4):
                            nc.tensor.matmul(
                                yts[a][:],
                                ATrep[32 * a : 32 * (a + 1), j2, h, :],
                                vv[32 * a : 32 * (a + 1), :, j2, :, h],
                                start=(j2 == 0 and h == 0),
                                stop=(j2 == 7 and h == 1),
                                tile_position=(32 * a, 0),
                            )

                # Taper supers: alternate drains onto the (by then idle)
                # Vector engine so the final drains run in parallel.
                use_vec = sg >= len(sizes) - 2 and sg % 2
                if contig:
                    # Undo the "(p t)" row permutation in the drain APs:
                    # row_local = 512*Tq + 128*a + 4*c + tr (Tq=T//4, tr=T%4)
                    ysb = y_pool.tile(
                        [128, nt // 4, 4, 32, 4], BF16, tag="ysb", name=f"ysb{sg}"
                    )
                    for a in range(4):
                        dst = ysb[:, :, a, :, :].rearrange("p q c t -> p q t c")
                        src = yts[a][:].rearrange("p (q t) c -> p q t c", t=4)
                        if use_vec:
                            nc.vector.tensor_copy(dst, src)
                        else:
                            nc.scalar.copy(dst, src)
                    out_src = ysb[:].rearrange("p q a c t -> p (q a c t)")
                else:
                    ysb = y_pool.tile(
                        [128, nt, 4, 32], BF16, tag="ysb", name=f"ysb{sg}"
                    )
                    for a in range(4):
                        if use_vec:
                            nc.vector.tensor_copy(ysb[:, :, a, :], yts[a][:])
                        else:
                            nc.scalar.copy(ysb[:, :, a, :], yts[a][:])
                    out_src = ysb[:].rearrange("p t a c -> p (t a c)")
                nc.sync.dma_start(out_d[:, rows], out_src)
                row0 += rows_n
            ps_cm.__exit__(None, None, None)

    nc.compile()
    return nc


_CACHE: dict = {}


def _get_nc():
    if "nc" not in _CACHE:
        _CACHE["nc"] = build_kernel()
    return _CACHE["nc"]


def _make_atrep(p_values):
    """atrep[32a + r, j2, h, p] = A[p, 64*j2 + 2r + h], A per the reference."""
    import ml_dtypes

    pv = np.asarray(p_values, dtype=np.float64).reshape(64, 1)
    m = np.arange(256, dtype=np.float64)
    ang = -pv * m * (2.0 * math.pi / 256.0)
    A_real, A_imag = np.cos(ang), np.sin(ang)       # [64, 256]
    A1 = np.concatenate((A_real, A_imag), axis=0)   # [128, 256]
    A2 = np.concatenate((-A_imag, A_real), axis=0)
    A = np.concatenate((A1, A2), axis=1)            # [128, 512]
    AT = A.T                                        # [512, 128] = AT[m, p]
    base = AT.reshape(8, 32, 2, 128).transpose(1, 0, 2, 3)  # [r, j2, h, p]
    rep = np.tile(base, (4, 1, 1, 1))               # [128, 8, 2, 128]
    return np.ascontiguousarray(rep.astype(ml_dtypes.bfloat16))


def _run(x, p_values, trace=False, **kw):
    nc = _get_nc()
    x = np.ascontiguousarray(x, dtype=np.float32)
    at = _make_atrep(p_values)
    in_maps = [
        {"x": x[c * SHARD : (c + 1) * SHARD].reshape(BN, M2), "at": at}
        for c in range(N_CORES)
    ]
    res = run_bass_kernel_spmd(
        nc, in_maps, core_ids=list(range(N_CORES)), trace=trace, **kw
    )
    out = np.empty((BATCH, NBLK * P2), dtype=np.float32)
    for c in range(N_CORES):
        # res is [128, 16384] bf16 = y_core^T; un-transpose during the gather
        out[c * SHARD : (c + 1) * SHARD] = (
            res.results[c]["out"].astype(np.float32).T.reshape(SHARD, NBLK * P2)
        )
    return out, res


def kernel(x, p_values):
    out, _ = _run(x, p_values)
    return out



# revision 51
# speedup vs baseline: 1.1137x; 1.0982x over previous
"""Trainium2 Bass kernel for nn_AnalogLayer.

Math (see reference):
    A[p, m] built from cos/sin of (-2*pi/256 * p_values[p%64] * (m%256)),
    y[bn, :] = A @ x[bn, :]  for each of the batch*32 rows of length 512.

Strategy: pure data-parallel over batch across 8 NeuronCores. Per core the
shard is viewed as [16384, 512] rows; output is produced transposed
([128, 16384] bf16) and fixed up on the host during the gather/unshard step.
The kernel is HBM-bound (33.5MB f32 in + 4.2MB bf16 out per core at ~358
GB/s/core ~= 105us); the design keeps the DMA stream saturated end-to-end.

Host side: A^T is precomputed from the tiny p_values and passed in
replicated per 32-row strip and even/odd split
(atrep[32a+r, j2, h, p] = A[p, 64*j2 + 2r + h]).

Per-core pipeline (per super-group of 2048 rows, in ~1MB chunks):
  - SWDGE cast-DMA x f32(HBM) -> bf16(SBUF) [128, 4, 512] chunks, with
    partition = row-block ("(p t)"): one contiguous 8KB read run per
    partition per chunk (4x fewer descriptors than the strided layout)
  - VectorE StreamTranspose of bf16 PAIRS viewed as f32 via bitcast
    (32x32 blocks of pairs): stream transpose moves 1 elem/lane/cycle
    regardless of dtype, so pairing halves DVE time (75us -> 41us).
    Partition 32a+r then holds m in {64*j2+2r, 64*j2+2r+1}.
  - 8 j2-blocks x 2 parities x 4 row-strips of K=32 matmuls
    (tile_position=(32a,0), quad-concurrent), lhsT = the matching
    even/odd A^T slice, rhs = stride-2 columns; same instruction count,
    N = nt*32 <= 512 per PSUM bank, one bank per strip
  - ScalarE drains PSUM -> SBUF bf16 with permuted APs (undoing the
    "(p t)" row permutation); out-DMA per super on the sync HWDGE ring
  - taper [2048x7, 1024, 512, 512] with final chunks [2,1,1] keeps the
    post-last-input serial chain (transpose+matmul+drain+out) short

Lessons encoded here (measured on HW): instruction count costs ~6-7ns
each in init TENSOR_LOAD + barrier + refill, so big-N matmuls matter;
fine-grained out-DMAs degrade the input stream's HBM efficiency; strided
matmul rhs APs collapse PE streaming; HWDGE-f32 + engine casts lose to
SWDGE cast-DMA; device timing has ~15% contention episodes (measure
min-of-5).
"""

import math

import numpy as np

import concourse.bacc as bacc
import concourse.bass as bass
import concourse.masks as masks
import concourse.mybir as mybir
import concourse.tile as tile
from concourse.bass_utils import run_bass_kernel_spmd

N_CORES = 8
BATCH = 4096
SHARD = BATCH // N_CORES  # 512 batch rows per core
NBLK = 32                 # blocks per batch row
M2 = 512                  # contraction dim (2*M)
P2 = 128                  # output dim (2*P)
BN = SHARD * NBLK         # 16384 logical rows per core
import os

SUPER = int(os.environ.get("K_SUPER", "2048"))  # rows per super-group
QN = int(os.environ.get("K_QN", "4"))           # t-tiles per load chunk
XFBUFS = int(os.environ.get("K_XFBUFS", "12"))
TAPER = os.environ.get("K_TAPER", "1") == "1"
NT = SUPER // 128
NSG = BN // SUPER
NJQ = 16                  # j-chunks (contraction 512 / 32)

F32 = mybir.dt.float32
BF16 = mybir.dt.bfloat16
PI = math.pi


def build_kernel():
    nc = bacc.Bacc("TRN2", target_bir_lowering=False)
    x_d = nc.declare_dram_parameter("x", [BN, M2], F32, isOutput=False)
    at_d = nc.declare_dram_parameter("at", [128, 8, 2, 128], BF16, isOutput=False)
    out_d = nc.declare_dram_parameter("out", [P2, BN], BF16, isOutput=True)

    with tile.TileContext(nc) as tc:
        with (
            tc.tile_pool(name="const", bufs=1) as cpool,
            tc.tile_pool(name="xf", bufs=2) as xf_pool,
            tc.tile_pool(name="vt", bufs=int(os.environ.get("K_VBUFS", "3"))) as vt_pool,
            tc.tile_pool(name="ysb", bufs=3) as y_pool,
        ):
            # A^T is host-precomputed (tiny: derived from the 64-entry
            # p_values) and passed in replicated and even/odd-split:
            # atrep[32a+r, j2, h, p] = A[p, 64*j2 + 2r + h]. The split
            # matches the f32-pair transpose trick (see main loop).
            ATrep = cpool.tile([128, 8, 2, 128], BF16)
            nc.scalar.dma_start(ATrep[:], at_d[:])

            # ---------------- main loop ----------------
            # Per super-group: SWDGE cast-DMA x f32->bf16 in ~1MB chunks ->
            # DVE 32x32 transpose per chunk -> 16j x 4a quad-32 matmuls with
            # N = nt*32 (large N amortizes LDWEIGHTS + keeps the instruction
            # count low; instruction refill rides DMA row 14) -> ScalarE
            # drains -> one coarse out-DMA per super on the sync ring
            # (fine-grained out writes interleave badly with the input read
            # stream at HBM). Single small final super keeps the tail short.
            ps_cm = tc.tile_pool(
                name="ps", bufs=(4 if SUPER == 4096 else 8), space="PSUM"
            )
            ps = ps_cm.__enter__()

            if SUPER == 4096 and TAPER:
                sizes = [4096] * 3 + [2048, 1024, 512, 512]
            elif SUPER == 2048 and TAPER:
                sizes = [2048] * 7 + [1024, 512, 512]
            else:
                sizes = [SUPER] * NSG
            assert sum(sizes) == BN
            # Full-size chunks in the body; the final supers use small
            # chunks so the DVE transpose backlog at input-end is at most
            # one ~0.3us instruction. (Instruction count is kept low: init
            # TENSOR_LOAD + barrier + refill cost ~6-7ns per instruction.)
            chunk_splits = {
                sg: ([4] * (r // 512) if r >= 512 else [r // 128])
                for sg, r in enumerate(sizes)
            }
            # Small chunks at the tail: the post-last-input chain is then
            # one small transpose + a short matmul phase + drain + out.
            if sizes[-1] == 512:
                chunk_splits[len(sizes) - 1] = [2, 1, 1]
            row0 = 0
            nchunk = 0
            for sg, rows_n in enumerate(sizes):
                nt = rows_n // 128
                rows = slice(row0, row0 + rows_n)
                # Body supers (uniform qn=4 chunks) load with partition =
                # row-block ("(p t)"): one contiguous 8KB run per partition
                # per chunk (4x fewer DMA descriptors, faster SWDGE
                # emission). This permutes rows within the chunk; the drain
                # APs undo it. Taper supers use the plain layout.
                contig = chunk_splits[sg] == [4] * (rows_n // 512)
                v = vt_pool.tile([128, nt, 512], BF16, tag="v", name=f"v{sg}")
                t0 = 0
                for qn in chunk_splits[sg]:
                    crows = slice(row0 + t0 * 128, row0 + (t0 + qn) * 128)
                    xf = xf_pool.tile(
                        [128, QN, 512], BF16, tag="xf", name=f"xf{sg}_{t0}",
                        bufs=XFBUFS,
                    )
                    if contig:
                        x_src = x_d[crows, :].rearrange("(p t) m -> p t m", t=qn)
                    else:
                        x_src = x_d[crows, :].rearrange("(t p) m -> p t m", p=128)
                    # SWDGE cast-DMA: f32 HBM read, bf16 SBUF write
                    nc.gpsimd.dma_start(xf[:, :qn, :], x_src)
                    # Transpose bf16 PAIRS as f32 elements: halves the DVE
                    # element count (stream transpose moves 1 elem/lane/cyc
                    # regardless of dtype). The pair interleave is undone
                    # by the even/odd split of A^T in the matmul.
                    nc.vector.transpose(
                        v[:, t0 : t0 + qn, :].bitcast(F32),
                        xf[:, :qn, :].bitcast(F32),
                    )
                    t0 += qn
                    nchunk += 1
                assert t0 == nt

                yts = [
                    ps.tile([128, nt, 32], F32, tag="yt", name=f"yt{sg}_{a}")
                    for a in range(4)
                ]
                # v[32a+r, t, 64*j2 + 2c + h] = x[row(a,c,t), 64*j2+2r+h]:
                # contract per (j2, h) with the matching A^T slice; columns
                # (t, c) land in the same output-row order as before.
                vv = v[:].rearrange("p t (j2 c h) -> p t j2 c h", c=32, h=2)
                for j2 in range(8):
                    for h in range(2):
                        for a in range(4):
                            nc.tensor.matmul(
                                yts[a][:],
                                ATrep[32 * a : 32 * (a + 1), j2, h, :],
                                vv[32 * a : 32 * (a + 1), :, j2, :, h],
                                start=(j2 == 0 and h == 0),
                                stop=(j2 == 7 and h == 1),
                                tile_position=(32 * a, 0),
                            )

                # Taper supers: alternate drains onto the (by then idle)
                # Vector engine so the final drains run in parallel.
                use_vec = sg >= len(sizes) - 2 and sg % 2
                if contig:
                    # Undo the "(p t)" row permutation in the drain APs:
                    # row_local = 512*Tq + 128*a + 4*c + tr (Tq=T//4, tr=T%4)
                    ysb = y_pool.tile(
                        [128, nt // 4, 4, 32, 4], BF16, tag="ysb", name=f"ysb{sg}"
                    )
                    for a in range(4):
                        dst = ysb[:, :, a, :, :].rearrange("p q c t -> p q t c")
                        src = yts[a][:].rearrange("p (q t) c -> p q t c", t=4)
                        if use_vec:
                            nc.vector.tensor_copy(dst, src)
                        else:
                            nc.scalar.copy(dst, src)
                    out_src = ysb[:].rearrange("p q a c t -> p (q a c t)")
                else:
                    ysb = y_pool.tile(
                        [128, nt, 4, 32], BF16, tag="ysb", name=f"ysb{sg}"
                    )
                    for a in range(4):
                        if use_vec:
                            nc.vector.tensor_copy(ysb[:, :, a, :], yts[a][:])
                        else:
                            nc.scalar.copy(ysb[:, :, a, :], yts[a][:])
                    out_src = ysb[:].rearrange("p t a c -> p (t a c)")
                nc.sync.dma_start(out_d[:, rows], out_src)
                row0 += rows_n
            ps_cm.__exit__(None, None, None)

    nc.compile()
    return nc


_CACHE: dict = {}


def _get_nc():
    if "nc" not in _CACHE:
        _CACHE["nc"] = build_kernel()
    return _CACHE["nc"]


def _make_atrep(p_values):
    """atrep[32a + r, j2, h, p] = A[p, 64*j2 + 2r + h], A per the reference."""
    import ml_dtypes

    pv = np.asarray(p_values, dtype=np.float64).reshape(64, 1)
    m = np.arange(256, dtype=np.float64)
    ang = -pv * m * (2.0 * math.pi / 256.0)
    A_real, A_imag = np.cos(ang), np.sin(ang)       # [64, 256]
    A1 = np.concatenate((A_real, A_imag), axis=0)   # [128, 256]
    A2 = np.concatenate((-A_imag, A_real), axis=0)
    A = np.concatenate((A1, A2), axis=1)            # [128, 512]
    AT = A.T                                        # [512, 128] = AT[m, p]
    base = AT.reshape(8, 32, 2, 128).transpose(1, 0, 2, 3)  # [r, j2, h, p]
    rep = np.tile(base, (4, 1, 1, 1))               # [128, 8, 2, 128]
    return np.ascontiguousarray(rep.astype(ml_dtypes.bfloat16))


def _run(x, p_values, trace=False, **kw):
    nc = _get_nc()
    x = np.ascontiguousarray(x, dtype=np.float32)
    at = _make_atrep(p_values)
    in_maps = [
        {"x": x[c * SHARD : (c + 1) * SHARD].reshape(BN, M2), "at": at}
        for c in range(N_CORES)
    ]
    res = run_bass_kernel_spmd(
        nc, in_maps, core_ids=list(range(N_CORES)), trace=trace, **kw
    )
    out = np.empty((BATCH, NBLK * P2), dtype=np.float32)
    for c in range(N_CORES):
        # res is [128, 16384] bf16 = y_core^T; un-transpose during the gather
        out[c * SHARD : (c + 1) * SHARD] = (
            res.results[c]["out"].astype(np.float32).T.reshape(SHARD, NBLK * P2)
        )
    return out, res


def kernel(x, p_values):
    out, _ = _run(x, p_values)
    return out



# revision 55
# speedup vs baseline: 1.1299x; 1.0145x over previous
"""Trainium2 Bass kernel for nn_AnalogLayer.

Math (see reference):
    A[p, m] built from cos/sin of (-2*pi/256 * p_values[p%64] * (m%256)),
    y[bn, :] = A @ x[bn, :]  for each of the batch*32 rows of length 512.

Strategy: pure data-parallel over batch across 8 NeuronCores. Per core the
shard is viewed as [16384, 512] rows; output is produced transposed
([128, 16384] bf16) and fixed up on the host during the gather/unshard step.
The kernel is HBM-bound (33.5MB f32 in + 4.2MB bf16 out per core at ~358
GB/s/core ~= 105us); the design keeps the DMA stream saturated end-to-end.

Host side: A^T is precomputed from the tiny p_values and passed in
replicated per 32-row strip and even/odd split
(atrep[32a+r, j2, h, p] = A[p, 64*j2 + 2r + h]).

Per-core pipeline (per super-group of 2048 rows, in ~1MB chunks):
  - SWDGE cast-DMA x f32(HBM) -> bf16(SBUF) [128, 4, 512] chunks, with
    partition = row-block ("(p t)"): one contiguous 8KB read run per
    partition per chunk (4x fewer descriptors than the strided layout)
  - VectorE StreamTranspose of bf16 PAIRS viewed as f32 via bitcast
    (32x32 blocks of pairs): stream transpose moves 1 elem/lane/cycle
    regardless of dtype, so pairing halves DVE time (75us -> 41us).
    Partition 32a+r then holds m in {64*j2+2r, 64*j2+2r+1}.
  - 8 j2-blocks x 2 parities x 4 row-strips of K=32 matmuls
    (tile_position=(32a,0), quad-concurrent), lhsT = the matching
    even/odd A^T slice, rhs = stride-2 columns; same instruction count,
    N = nt*32 <= 512 per PSUM bank, one bank per strip
  - ScalarE drains PSUM -> SBUF bf16 with permuted APs (undoing the
    "(p t)" row permutation); out-DMA per super on the sync HWDGE ring
  - taper [2048x7, 1024, 512, 512] with final chunks [2,1,1] keeps the
    post-last-input serial chain (transpose+matmul+drain+out) short

Lessons encoded here (measured on HW): instruction count costs ~6-7ns
each in init TENSOR_LOAD + barrier + refill, so big-N matmuls matter;
fine-grained out-DMAs degrade the input stream's HBM efficiency; strided
matmul rhs APs collapse PE streaming; HWDGE-f32 + engine casts lose to
SWDGE cast-DMA; device timing has ~15% contention episodes (measure
min-of-5).
"""

import math

import numpy as np

import concourse.bacc as bacc
import concourse.bass as bass
import concourse.masks as masks
import concourse.mybir as mybir
import concourse.tile as tile
from concourse.bass_utils import run_bass_kernel_spmd

N_CORES = 8
BATCH = 4096
SHARD = BATCH // N_CORES  # 512 batch rows per core
NBLK = 32                 # blocks per batch row
M2 = 512                  # contraction dim (2*M)
P2 = 128                  # output dim (2*P)
BN = SHARD * NBLK         # 16384 logical rows per core
import os

SUPER = int(os.environ.get("K_SUPER", "2048"))  # rows per super-group
QN = int(os.environ.get("K_QN", "4"))           # t-tiles per load chunk
XFBUFS = int(os.environ.get("K_XFBUFS", "12"))
TAPER = os.environ.get("K_TAPER", "1") == "1"
NT = SUPER // 128
NSG = BN // SUPER
NJQ = 16                  # j-chunks (contraction 512 / 32)

F32 = mybir.dt.float32
BF16 = mybir.dt.bfloat16
PI = math.pi


def build_kernel():
    nc = bacc.Bacc("TRN2", target_bir_lowering=False)
    x_d = nc.declare_dram_parameter("x", [BN, M2], F32, isOutput=False)
    at_d = nc.declare_dram_parameter("at", [128, 8, 2, 128], BF16, isOutput=False)
    out_d = nc.declare_dram_parameter("out", [P2, BN], BF16, isOutput=True)

    with tile.TileContext(nc) as tc:
        with (
            tc.tile_pool(name="const", bufs=1) as cpool,
            tc.tile_pool(name="xf", bufs=2) as xf_pool,
            tc.tile_pool(name="vt", bufs=int(os.environ.get("K_VBUFS", "3"))) as vt_pool,
            tc.tile_pool(name="ysb", bufs=3) as y_pool,
        ):
            # A^T is host-precomputed (tiny: derived from the 64-entry
            # p_values) and passed in replicated and even/odd-split:
            # atrep[32a+r, j2, h, p] = A[p, 64*j2 + 2r + h]. The split
            # matches the f32-pair transpose trick (see main loop).
            ATrep = cpool.tile([128, 8, 2, 128], BF16)
            nc.scalar.dma_start(ATrep[:], at_d[:])

            # ---------------- main loop ----------------
            # Per super-group: SWDGE cast-DMA x f32->bf16 in ~1MB chunks ->
            # DVE 32x32 transpose per chunk -> 16j x 4a quad-32 matmuls with
            # N = nt*32 (large N amortizes LDWEIGHTS + keeps the instruction
            # count low; instruction refill rides DMA row 14) -> ScalarE
            # drains -> one coarse out-DMA per super on the sync ring
            # (fine-grained out writes interleave badly with the input read
            # stream at HBM). Single small final super keeps the tail short.
            ps_cm = tc.tile_pool(
                name="ps", bufs=(4 if SUPER == 4096 else 8), space="PSUM"
            )
            ps = ps_cm.__enter__()

            if SUPER == 4096 and TAPER:
                sizes = [4096] * 3 + [2048, 1024, 512, 512]
            elif SUPER == 2048 and TAPER:
                sizes = [2048] * 7 + [1024, 512, 512]
            else:
                sizes = [SUPER] * NSG
            assert sum(sizes) == BN
            # Full-size chunks in the body; the final supers use small
            # chunks so the DVE transpose backlog at input-end is at most
            # one ~0.3us instruction. (Instruction count is kept low: init
            # TENSOR_LOAD + barrier + refill cost ~6-7ns per instruction.)
            chunk_splits = {
                sg: (
                    [QN] * (r // (128 * QN))
                    if r % (128 * QN) == 0
                    else [4] * (r // 512) if r >= 512 else [r // 128]
                )
                for sg, r in enumerate(sizes)
            }
            # Small chunks at the tail: the post-last-input chain is then
            # one small transpose + a short matmul phase + drain + out.
            if sizes[-1] == 512:
                chunk_splits[len(sizes) - 1] = [2, 1, 1]
            row0 = 0
            nchunk = 0
            for sg, rows_n in enumerate(sizes):
                nt = rows_n // 128
                rows = slice(row0, row0 + rows_n)
                # Body supers (uniform qn=4 chunks) load with partition =
                # row-block ("(p t)"): one contiguous 8KB run per partition
                # per chunk (4x fewer DMA descriptors, faster SWDGE
                # emission). This permutes rows within the chunk; the drain
                # APs undo it. Taper supers use the plain layout.
                contig = chunk_splits[sg] == [QN] * (rows_n // (128 * QN))
                v = vt_pool.tile([128, nt, 512], BF16, tag="v", name=f"v{sg}")
                t0 = 0
                for qn in chunk_splits[sg]:
                    crows = slice(row0 + t0 * 128, row0 + (t0 + qn) * 128)
                    xf = xf_pool.tile(
                        [128, QN, 512], BF16, tag="xf", name=f"xf{sg}_{t0}",
                        bufs=XFBUFS,
                    )
                    if contig:
                        x_src = x_d[crows, :].rearrange("(p t) m -> p t m", t=qn)
                    else:
                        x_src = x_d[crows, :].rearrange("(t p) m -> p t m", p=128)
                    # SWDGE cast-DMA: f32 HBM read, bf16 SBUF write
                    nc.gpsimd.dma_start(xf[:, :qn, :], x_src)
                    # Transpose bf16 PAIRS as f32 elements: halves the DVE
                    # element count (stream transpose moves 1 elem/lane/cyc
                    # regardless of dtype). The pair interleave is undone
                    # by the even/odd split of A^T in the matmul.
                    nc.vector.transpose(
                        v[:, t0 : t0 + qn, :].bitcast(F32),
                        xf[:, :qn, :].bitcast(F32),
                    )
                    t0 += qn
                    nchunk += 1
                assert t0 == nt

                yts = [
                    ps.tile([128, nt, 32], F32, tag="yt", name=f"yt{sg}_{a}")
                    for a in range(4)
                ]
                # v[32a+r, t, 64*j2 + 2c + h] = x[row(a,c,t), 64*j2+2r+h]:
                # contract per (j2, h) with the matching A^T slice; columns
                # (t, c) land in the same output-row order as before.
                vv = v[:].rearrange("p t (j2 c h) -> p t j2 c h", c=32, h=2)
                for j2 in range(8):
                    for h in range(2):
                        for a in range(4):
                            nc.tensor.matmul(
                                yts[a][:],
                                ATrep[32 * a : 32 * (a + 1), j2, h, :],
                                vv[32 * a : 32 * (a + 1), :, j2, :, h],
                                start=(j2 == 0 and h == 0),
                                stop=(j2 == 7 and h == 1),
                                tile_position=(32 * a, 0),
                            )

                # Taper supers: alternate drains onto the (by then idle)
                # Vector engine so the final drains run in parallel.
                use_vec = sg >= len(sizes) - 2 and sg % 2
                if contig:
                    # Undo the "(p t)" row permutation in the drain APs:
                    # row_local = 128*QN*Tq + 32*QN*a + QN*c + tr
                    ysb = y_pool.tile(
                        [128, nt // QN, 4, 32, QN], BF16, tag="ysb", name=f"ysb{sg}"
                    )
                    for a in range(4):
                        dst = ysb[:, :, a, :, :].rearrange("p q c t -> p q t c")
                        src = yts[a][:].rearrange("p (q t) c -> p q t c", t=QN)
                        if use_vec:
                            nc.vector.tensor_copy(dst, src)
                        else:
                            nc.scalar.copy(dst, src)
                    out_src = ysb[:].rearrange("p q a c t -> p (q a c t)")
                else:
                    ysb = y_pool.tile(
                        [128, nt, 4, 32], BF16, tag="ysb", name=f"ysb{sg}"
                    )
                    for a in range(4):
                        if use_vec:
                            nc.vector.tensor_copy(ysb[:, :, a, :], yts[a][:])
                        else:
                            nc.scalar.copy(ysb[:, :, a, :], yts[a][:])
                    out_src = ysb[:].rearrange("p t a c -> p (t a c)")
                nc.sync.dma_start(out_d[:, rows], out_src)
                row0 += rows_n
            ps_cm.__exit__(None, None, None)

    nc.compile()
    return nc


_CACHE: dict = {}


def _get_nc():
    if "nc" not in _CACHE:
        _CACHE["nc"] = build_kernel()
    return _CACHE["nc"]


def _make_atrep(p_values):
    """atrep[32a + r, j2, h, p] = A[p, 64*j2 + 2r + h], A per the reference."""
    import ml_dtypes

    pv = np.asarray(p_values, dtype=np.float64).reshape(64, 1)
    m = np.arange(256, dtype=np.float64)
    ang = -pv * m * (2.0 * math.pi / 256.0)
    A_real, A_imag = np.cos(ang), np.sin(ang)       # [64, 256]
    A1 = np.concatenate((A_real, A_imag), axis=0)   # [128, 256]
    A2 = np.concatenate((-A_imag, A_real), axis=0)
    A = np.concatenate((A1, A2), axis=1)            # [128, 512]
    AT = A.T                                        # [512, 128] = AT[m, p]
    base = AT.reshape(8, 32, 2, 128).transpose(1, 0, 2, 3)  # [r, j2, h, p]
    rep = np.tile(base, (4, 1, 1, 1))               # [128, 8, 2, 128]
    return np.ascontiguousarray(rep.astype(ml_dtypes.bfloat16))


def _run(x, p_values, trace=False, **kw):
    nc = _get_nc()
    x = np.ascontiguousarray(x, dtype=np.float32)
    at = _make_atrep(p_values)
    in_maps = [
        {"x": x[c * SHARD : (c + 1) * SHARD].reshape(BN, M2), "at": at}
        for c in range(N_CORES)
    ]
    res = run_bass_kernel_spmd(
        nc, in_maps, core_ids=list(range(N_CORES)), trace=trace, **kw
    )
    out = np.empty((BATCH, NBLK * P2), dtype=np.float32)
    for c in range(N_CORES):
        # res is [128, 16384] bf16 = y_core^T; un-transpose during the gather
        out[c * SHARD : (c + 1) * SHARD] = (
            res.results[c]["out"].astype(np.float32).T.reshape(SHARD, NBLK * P2)
        )
    return out, res


def kernel(x, p_values):
    out, _ = _run(x, p_values)
    return out

